# revision 55
# baseline (speedup 1.0000x reference)
"""GQA kernel for Trainium2, 8 NeuronCores.

Sharding: (batch x kv-head) — cores 0-3 handle batch 0, cores 4-7 batch 1;
each core owns 2 KV heads (8 Q heads). Row-parallel Wo via AllGather of the
attention outputs (bf16) within each 4-core group; each core computes a
512-column slice of the final output.

All layout transposes (x, Wq/Wk/Wv/Wo, mask exp) are done host-side in
numpy, so the device only runs useful matmuls. The Tensor engine is kept
continuously busy (p-state ramp): all QKV projections run first as one
dense block, attention scores are emitted one kv-tile ahead of the AV
accumulation, softmax normalization broadcasts the reciprocal row-sum via a
tiny K=2 PE matmul (no DRAM round-trip), and the AllGather is split into 4
per-s-chunk collectives overlapped with the next chunk's attention.

B=2, S=2048, H=2048, NH=32, NKV=8, HD=64. All matmuls bf16 (f32 PSUM).
Causal structure exploited: fully-masked upper tiles skipped; diagonal
tiles masked multiplicatively with host-computed exp(mask).
"""
import numpy as np
import ml_dtypes

import concourse.bass as bass
import concourse.tile as tile
from concourse import library_config, mybir
from concourse.bass_utils import run_bass_kernel_spmd

B, S, H = 2, 2048, 2048
NH, NKV, HD = 32, 8, 64
SCALE = HD ** -0.5
F32 = mybir.dt.float32
BF16 = mybir.dt.bfloat16
BF16_NP = ml_dtypes.bfloat16

_program_cache = {}
_trace_opts = {}       # test.py may set {"trace": True, "trace_cores": [...], "tmpdir": ...}
_last_results = None   # BassKernelResults of the most recent kernel() call


def _build_program():
    nc = bass.Bass("TRN2", target_bir_lowering=False, debug=False, num_devices=8)

    xT_in = nc.dram_tensor("xT", [H, S], BF16, kind="ExternalInput").ap()
    em_in = nc.dram_tensor("em", [128, 16, 512], BF16, kind="ExternalInput").ap()
    wq_in = nc.dram_tensor("wqT", [H, 512], BF16, kind="ExternalInput").ap()
    wk_in = nc.dram_tensor("wkT", [H, 128], BF16, kind="ExternalInput").ap()
    wv_in = nc.dram_tensor("wvT", [H, 128], BF16, kind="ExternalInput").ap()
    wo_in = nc.dram_tensor("woT", [H, 512], BF16, kind="ExternalInput").ap()
    out_ext = nc.dram_tensor("out_part", [S, 512], F32, kind="ExternalOutput").ap()

    with tile.TileContext(nc) as tc:
        import contextlib
        with (
            tc.tile_pool(name="persist", bufs=1) as persist,
            tc.tile_pool(name="dram", bufs=1, space="DRAM") as dram,
        ):
            cc_in = [dram.tile([256, 512], BF16, name=f"cc_in_{u}") for u in range(8)]
            cc_out = [dram.tile([1024, 512], BF16, name=f"cc_out_{u}") for u in range(8)]
            warm_in = dram.tile([128, 512], BF16)
            warm_out = dram.tile([512, 512], BF16)
            rc_dram = dram.tile([32, 512], F32)   # recip bounce: [2*(4c+m)+half, q]

            # ---- persistent sbuf ----
            wq_sb = persist.tile([128, 16, 512], BF16)   # [h_in, h_chunk, qd]
            wk_sb = persist.tile([128, 16, 128], BF16)
            wv_sb = persist.tile([128, 16, 128], BF16)
            wo_sb = persist.tile([128, 16, 512], BF16)   # [d_in, d_chunk, hcol]
            em_sb = persist.tile([128, 16, 512], BF16)   # exp(mask)^T per kv tile
            qt_sb = persist.tile([128, 4, S], BF16)      # [qd in pair, pair m, s]
            kt_sb = persist.tile([128, S], BF16)         # [d (2 heads), skv]
            v_sb = persist.tile([128, 16, 130], BF16)    # [skv in tile, t, V0|1|V1|1]
            nc.vector.memset(v_sb, 1.0)

            # warm up the collective rings with a real-size AllGather
            warm_sb = persist.tile([128, 512], BF16)
            nc.vector.memset(warm_sb, 0.0)
            nc.gpsimd.dma_start(out=warm_in, in_=warm_sb[:])
            nc.gpsimd.collective_compute(
                "AllGather", mybir.AluOpType.bypass,
                replica_groups=[[0, 1, 2, 3], [4, 5, 6, 7]],
                ins=[warm_in.opt()], outs=[warm_out.opt()])

            # ---- phase P: all projections, dense on PE ----
            # xt lives only in this scope; its 64KB/partition is reused by
            # the phase A/O pools afterwards.
            with (
                tc.tile_pool(name="xtpool", bufs=1) as xtpool,
                tc.tile_pool(name="proj_ps", bufs=3, space="PSUM") as proj_ps,
                tc.tile_pool(name="v_ps", bufs=2, space="PSUM") as v_ps,
            ):
                xt = [xtpool.tile([128, S], BF16, name=f"xt_{i}") for i in range(16)]
                # input DMAs split across both HWDGE queues
                nc.scalar.dma_start(out=wq_sb,
                                    in_=wq_in.rearrange("(i p) d -> p i d", p=128))
                for i in range(16):
                    eng = (nc.sync, nc.scalar, nc.gpsimd)[i % 3]
                    eng.dma_start(out=xt[i], in_=xT_in[128 * i:128 * (i + 1), :])
                nc.sync.dma_start(out=wk_sb,
                                  in_=wk_in.rearrange("(i p) d -> p i d", p=128))
                nc.sync.dma_start(out=wv_sb,
                                  in_=wv_in.rearrange("(i p) d -> p i d", p=128))
                nc.scalar.dma_start(out=em_sb, in_=em_in)
                nc.sync.dma_start(out=wo_sb,
                                  in_=wo_in.rearrange("(i p) d -> p i d", p=128))
                for c in range(4):
                    cs = slice(512 * c, 512 * (c + 1))
                    for m in range(4):
                        qp = proj_ps.tile([128, 512], F32, tag="pps", name=f"qp_{c}_{m}")
                        for i in range(16):
                            nc.tensor.matmul(qp[:], wq_sb[:, i, 128 * m:128 * (m + 1)],
                                             xt[i][:, cs], start=(i == 0), stop=(i == 15))
                        nc.vector.tensor_copy(qt_sb[:, m, cs], qp[:])
                    kp = proj_ps.tile([128, 512], F32, tag="pps", name=f"kp_{c}")
                    for i in range(16):
                        nc.tensor.matmul(kp[:], wk_sb[:, i, :], xt[i][:, cs],
                                         start=(i == 0), stop=(i == 15))
                    nc.vector.tensor_copy(kt_sb[:, cs], kp[:])
                    for r in range(4):
                        t = 4 * c + r
                        vp = v_ps.tile([128, 128], F32, tag="vps", name=f"vp_{t}")
                        for i in range(16):
                            nc.tensor.matmul(vp[:], xt[i][:, 128 * t:128 * (t + 1)],
                                             wv_sb[:, i, :], start=(i == 0), stop=(i == 15))
                        nc.vector.tensor_copy(v_sb[:, t, 0:64], vp[:, 0:64])
                        nc.vector.tensor_copy(v_sb[:, t, 65:129], vp[:, 64:128])

            # ---- phase A + O: attention, chunked collective, output proj ----
            with contextlib.ExitStack() as ctx:
                sc_ps = ctx.enter_context(tc.tile_pool(name="sc_ps", bufs=2, space="PSUM"))
                av_ps = ctx.enter_context(tc.tile_pool(name="av_ps", bufs=2, space="PSUM"))
                o_ps = ctx.enter_context(tc.tile_pool(name="o_ps", bufs=2, space="PSUM"))
                bc_pool = ctx.enter_context(tc.tile_pool(name="bc", bufs=2))
                probs = ctx.enter_context(tc.tile_pool(name="probs", bufs=4))
                smalls = ctx.enter_context(tc.tile_pool(name="smalls", bufs=2))
                tmp_pool = ctx.enter_context(tc.tile_pool(name="tmp", bufs=2))
                ot_pool = ctx.enter_context(tc.tile_pool(name="ot", bufs=2))
                of_pool = ctx.enter_context(tc.tile_pool(name="of", bufs=2))
                outst = ctx.enter_context(tc.tile_pool(name="outst", bufs=2))

                partial_pool = ctx.enter_context(tc.tile_pool(name="partial", bufs=2))

                # filler FIFO: O-projection PE work pumped into the exp-wait
                # bubbles of the attention tile loop (keeps HAM warm)
                filler = []

                def pump(n):
                    for _ in range(n):
                        if filler:
                            filler.pop(0)()

                def enqueue_oproj_half(c, of_tile, h2, partial):
                    """32 matmuls (+tails) computing the i in [8*h2, 8*h2+8)
                    half of chunk c's output projection."""
                    state = {}
                    for st in range(4):
                        for k in range(8):
                            def mm(st=st, k=k, i=8 * h2 + 0 + 0):
                                i = 8 * h2 + k
                                if k == 0:
                                    state[st] = o_ps.tile(
                                        [128, 512], F32, tag="ops",
                                        name=f"op_{c}_{h2}_{st}")
                                nc.tensor.matmul(
                                    state[st][:],
                                    of_tile[:, i, 128 * st:128 * (st + 1)],
                                    wo_sb[:, i, :],
                                    start=(k == 0), stop=(k == 7))
                            filler.append(mm)

                        if h2 == 0:
                            def tail0(st=st):
                                nc.vector.tensor_copy(partial[:, st, :],
                                                      state[st][:])
                            filler.append(tail0)
                        else:
                            def tail1(st=st):
                                ost = outst.tile([128, 512], F32, tag="ost",
                                                 name=f"ost_{c}_{st}")
                                nc.vector.tensor_tensor(
                                    ost[:], state[st][:], partial[:, st, :],
                                    op=mybir.AluOpType.add)
                                nc.gpsimd.dma_start(
                                    out=out_ext[512 * c + 128 * st:
                                                512 * c + 128 * (st + 1), :],
                                    in_=ost[:])
                            filler.append(tail1)

                def emit_scores(c, m, t):
                    """Scores + fused-pair exp (+ diag mask) for kv tile t."""
                    cs = slice(512 * c, 512 * (c + 1))
                    sp = sc_ps.tile([128, 1024], F32, tag="sp2",
                                    name=f"sp_{c}_{m}_{t}")
                    for half in (0, 1):
                        hp = slice(64 * half, 64 * half + 64)
                        nc.tensor.matmul(sp[:, 512 * half:512 * (half + 1)],
                                         kt_sb[hp, 128 * t:128 * (t + 1)],
                                         qt_sb[hp, m, cs], start=True, stop=True,
                                         tile_position=(64 * half, 0))
                    pr = probs.tile([128, 1024], BF16, tag="pr",
                                    name=f"pr_{c}_{m}_{t}")
                    nc.scalar.activation(pr[:], sp[:],
                                         mybir.ActivationFunctionType.Exp,
                                         scale=SCALE)
                    if t >= 4 * c:
                        nc.vector.tensor_mul(pr[:, 0:512], pr[:, 0:512],
                                             em_sb[:, t, :])
                        nc.vector.tensor_mul(pr[:, 512:1024], pr[:, 512:1024],
                                             em_sb[:, t, :])
                    return pr

                def emit_av(avA, avB, pr, t, first, last):
                    nc.tensor.matmul(avA[:], v_sb[:, t, 0:65], pr[:, 0:512],
                                     start=first, stop=last)
                    nc.tensor.matmul(avB[:], v_sb[:, t, 65:130], pr[:, 512:1024],
                                     start=first, stop=last)

                def attention_chunk(c, ot_tile, of_tile, enq):
                    for m in range(4):
                        for fn in enq.get(m, []):
                            fn()
                        ntile = 4 * c + 4
                        avA = av_ps.tile([65, 512], F32, tag="av", name=f"avA_{c}_{m}")
                        avB = av_ps.tile([65, 512], F32, tag="av", name=f"avB_{c}_{m}")
                        pend = None
                        for t in range(ntile):
                            pr = emit_scores(c, m, t)
                            if pend is not None:
                                pt, ppr = pend
                                emit_av(avA, avB, ppr, pt, pt == 0, False)
                            pend = (t, pr)
                            pump(2)
                        pt, ppr = pend
                        emit_av(avA, avB, ppr, pt, pt == 0, True)

                        # normalization, all off the PE/scalar critical path:
                        # copy av + sum rows out (vector), DMA-bounce broadcast
                        # the sums, then divide on gpsimd
                        sums = smalls.tile([33, 512], F32, tag="rc",
                                           name=f"sums_{c}_{m}")
                        tmp = tmp_pool.tile([128, 512], F32, tag="tmp",
                                            name=f"tmp_{c}_{m}")
                        bc = bc_pool.tile([128, 512], F32, tag="bc",
                                          name=f"bc_{c}_{m}")
                        rcp = smalls.tile([33, 512], F32, tag="rcp",
                                          name=f"rcp_{c}_{m}")
                        nc.vector.tensor_copy(sums[0:1, :], avA[64:65, :])
                        nc.vector.tensor_copy(sums[32:33, :], avB[64:65, :])
                        nc.vector.reciprocal(rcp[:], sums[:])
                        nc.vector.tensor_copy(tmp[0:64, :], avA[0:64, :])
                        nc.vector.tensor_copy(tmp[64:128, :], avB[0:64, :])
                        u = 2 * (4 * c + m)
                        nc.gpsimd.dma_start(out=rc_dram[u:u + 1, :], in_=rcp[0:1, :])
                        nc.gpsimd.dma_start(out=rc_dram[u + 1:u + 2, :],
                                            in_=rcp[32:33, :])
                        nc.gpsimd.dma_start(
                            out=bc[0:64, :],
                            in_=rc_dram[u:u + 1, :].partition_broadcast(64))
                        nc.gpsimd.dma_start(
                            out=bc[64:128, :],
                            in_=rc_dram[u + 1:u + 2, :].partition_broadcast(64))
                        nc.gpsimd.tensor_mul(ot_tile[:, m, :], tmp[:], bc[:])

                        # per-half-chunk collective: gather these 2 head
                        # pairs' OT across the group, load into of
                        if m in (1, 3):
                            h2 = m // 2
                            uu = 2 * c + h2
                            nc.gpsimd.dma_start(
                                out=cc_in[uu].rearrange("(mm p) s -> p mm s", p=128),
                                in_=ot_tile[:, 2 * h2:2 * (h2 + 1), :])
                            nc.gpsimd.collective_compute(
                                "AllGather", mybir.AluOpType.bypass,
                                replica_groups=[[0, 1, 2, 3], [4, 5, 6, 7]],
                                ins=[cc_in[uu].opt()], outs=[cc_out[uu].opt()])
                            nc.sync.dma_start(
                                out=of_tile[:, 8 * h2:8 * (h2 + 1), :],
                                in_=cc_out[uu].rearrange("(r p) s -> p r s", p=128))

                ctxs = {}
                for c in range(4):
                    ot_tile = ot_pool.tile([128, 4, 512], BF16, tag="ot", name=f"ot_{c}")
                    of_tile = of_pool.tile([128, 16, 512], BF16, tag="of",
                                           name=f"of_{c}")
                    partial = partial_pool.tile([128, 4, 512], F32, tag="part",
                                                name=f"part_{c}")
                    enq = {}
                    if c == 1:
                        enq.setdefault(0, []).append(
                            lambda p=ctxs[0]: enqueue_oproj_half(0, p[0], 0, p[1]))
                    if c >= 1:
                        enq.setdefault(1, []).append(
                            lambda p=ctxs[c - 1], cc=c - 1:
                            enqueue_oproj_half(cc, p[0], 1, p[1]))
                        enq.setdefault(3, []).append(
                            lambda cc=c, o=of_tile, pa=partial:
                            enqueue_oproj_half(cc, o, 0, pa))
                    attention_chunk(c, ot_tile, of_tile, enq)
                    ctxs[c] = (of_tile, partial)
                pump(len(filler))
                enqueue_oproj_half(3, ctxs[3][0], 1, ctxs[3][1])
                pump(len(filler))

    _split_excess_waits(nc)
    return nc


def _split_excess_waits(nc, cap=1):
    """Walrus allows few sync-wait slots per instruction; move excess waits
    onto same-engine NoOps placed immediately before (program order keeps
    semantics)."""
    nid = [0]
    for fn in nc.m.functions:
        for bb in fn.blocks:
            insts = list(bb.instructions)
            out = []
            for inst in insts:
                si = getattr(inst, "sync_info", None)
                waits = list(si.on_wait) if si and si.on_wait else []
                if len(waits) > cap:
                    keep = waits[:cap]
                    rest = waits[cap:]
                    while rest:
                        chunk, rest = rest[:cap], rest[cap:]
                        nid[0] += 1
                        nop = mybir.InstNoOp(
                            name=f"waitsplit-{nid[0]}", engine=inst.engine,
                            ins=[], outs=[], bass_nofuse=True,
                            sync_info=mybir.SyncInfo(on_wait=chunk, on_update=[]))
                        out.append(nop)
                    si.on_wait = keep
                out.append(inst)
            bb.instructions[:] = out


def _perm():
    """wo column permutation: gathered-OT row g -> original Wo column.

    Per-half-chunk AllGather: g = 1024*h2 + 256*rank + 128*mm + 64*half + d,
    head pair m = 2*h2 + mm."""
    p = np.zeros(2048, np.int64)
    for g in range(2048):
        h2, r1 = divmod(g, 1024)
        rank, r2 = divmod(r1, 256)
        mm, rr = divmod(r2, 128)
        half, d = divmod(rr, 64)
        h = 8 * rank + 2 * h2 + mm + 4 * half
        p[g] = 64 * h + d
    return p


def kernel(hidden_states, attention_mask, Wq, Wk, Wv, Wo):
    hidden_states = np.asarray(hidden_states, np.float32)
    attention_mask = np.asarray(attention_mask, np.float32)
    Wq = np.asarray(Wq, np.float32); Wk = np.asarray(Wk, np.float32)
    Wv = np.asarray(Wv, np.float32); Wo = np.asarray(Wo, np.float32)

    if "nc" not in _program_cache:
        _program_cache["nc"] = _build_program()
    nc = _program_cache["nc"]

    perm = _perm()
    # per-batch shared tensors
    xTs, ems = [], []
    for bi in range(B):
        xTs.append(np.ascontiguousarray(hidden_states[bi].T).astype(BF16_NP))
        em = np.zeros((128, 16, 512), np.float32)
        for c in range(4):
            blk = attention_mask[bi, 0, 512 * c:512 * (c + 1), 512 * c:512 * (c + 1)]
            emb = np.exp(blk).T  # [kv, q]
            for dd in range(4):
                em[:, 4 * c + dd, :] = emb[128 * dd:128 * (dd + 1), :]
        ems.append(em.astype(BF16_NP))

    wqTs, wkTs, wvTs, woTs = [], [], [], []
    for j in range(4):
        rows = []
        for m in range(4):
            rows.append(Wq[64 * (8 * j + m):64 * (8 * j + m) + 64])
            rows.append(Wq[64 * (8 * j + 4 + m):64 * (8 * j + 4 + m) + 64])
        wq_core = np.concatenate(rows, 0)
        wqTs.append(np.ascontiguousarray(wq_core.T).astype(BF16_NP))
        wkTs.append(np.ascontiguousarray(Wk[128 * j:128 * (j + 1)].T).astype(BF16_NP))
        wvTs.append(np.ascontiguousarray(Wv[128 * j:128 * (j + 1)].T).astype(BF16_NP))
        wo_core = Wo[512 * j:512 * (j + 1)][:, perm]
        woTs.append(np.ascontiguousarray(wo_core.T).astype(BF16_NP))

    in_maps = []
    for core in range(8):
        bi, j = divmod(core, 4)
        in_maps.append({
            "xT": xTs[bi], "em": ems[bi],
            "wqT": wqTs[j], "wkT": wkTs[j], "wvT": wvTs[j], "woT": woTs[j],
        })

    global _last_results
    res = run_bass_kernel_spmd(nc, in_maps, list(range(8)), **_trace_opts)
    _last_results = res
    out = np.zeros((B, S, H), np.float32)
    for core in range(8):
        bi, j = divmod(core, 4)
        out[bi, :, 512 * j:512 * (j + 1)] = res.results[core]["out_part"]
    return out


if __name__ == "__main__":
    ins = {
        "hidden_states": np.random.randn(B, S, H).astype(np.float32),
        "attention_mask": np.zeros((B, 1, S, S), np.float32),
        "Wq": np.random.randn(2048, H).astype(np.float32) * H ** -0.5,
        "Wk": np.random.randn(512, H).astype(np.float32) * H ** -0.5,
        "Wv": np.random.randn(512, H).astype(np.float32) * H ** -0.5,
        "Wo": np.random.randn(H, 2048).astype(np.float32) * H ** -0.5,
    }
    o = kernel(**ins)
    print("ran", o.shape, o.dtype)


# revision 60
# speedup vs baseline: 1.0929x; 1.0929x over previous
"""GQA kernel for Trainium2, 8 NeuronCores.

Sharding: (batch x kv-head) — cores 0-3 handle batch 0, cores 4-7 batch 1;
each core owns 2 KV heads (8 Q heads). Row-parallel Wo via AllGather of the
attention outputs (bf16) within each 4-core group; each core computes a
512-column slice of the final output.

All layout transposes (x, Wq/Wk/Wv/Wo, mask exp) are done host-side in
numpy, so the device only runs useful matmuls. The Tensor engine is kept
continuously busy (p-state ramp): all QKV projections run first as one
dense block, attention scores are emitted one kv-tile ahead of the AV
accumulation, softmax normalization broadcasts the reciprocal row-sum via a
tiny K=2 PE matmul (no DRAM round-trip), and the AllGather is split into 4
per-s-chunk collectives overlapped with the next chunk's attention.

B=2, S=2048, H=2048, NH=32, NKV=8, HD=64. All matmuls bf16 (f32 PSUM).
Causal structure exploited: fully-masked upper tiles skipped; diagonal
tiles masked multiplicatively with host-computed exp(mask).
"""
import numpy as np
import ml_dtypes

import concourse.bass as bass
import concourse.tile as tile
from concourse import library_config, mybir
from concourse.bass_utils import run_bass_kernel_spmd

B, S, H = 2, 2048, 2048
NH, NKV, HD = 32, 8, 64
SCALE = HD ** -0.5
F32 = mybir.dt.float32
BF16 = mybir.dt.bfloat16
BF16_NP = ml_dtypes.bfloat16

_program_cache = {}
_trace_opts = {}       # test.py may set {"trace": True, "trace_cores": [...], "tmpdir": ...}
_last_results = None   # BassKernelResults of the most recent kernel() call


def _build_program():
    nc = bass.Bass("TRN2", target_bir_lowering=False, debug=False, num_devices=8)

    xT_in = nc.dram_tensor("xT", [H, S], BF16, kind="ExternalInput").ap()
    em_in = nc.dram_tensor("em", [128, 16, 512], BF16, kind="ExternalInput").ap()
    wq_in = nc.dram_tensor("wqT", [H, 512], BF16, kind="ExternalInput").ap()
    wk_in = nc.dram_tensor("wkT", [H, 128], BF16, kind="ExternalInput").ap()
    wv_in = nc.dram_tensor("wvT", [H, 128], BF16, kind="ExternalInput").ap()
    wo_in = nc.dram_tensor("woT", [H, 512], BF16, kind="ExternalInput").ap()
    out_ext = nc.dram_tensor("out_part", [S, 512], F32, kind="ExternalOutput").ap()

    with tile.TileContext(nc) as tc:
        import contextlib
        with (
            tc.tile_pool(name="persist", bufs=1) as persist,
            tc.tile_pool(name="dram", bufs=1, space="DRAM") as dram,
        ):
            cc_in = [dram.tile([256, 512], BF16, name=f"cc_in_{u}") for u in range(8)]
            cc_out = [dram.tile([1024, 512], BF16, name=f"cc_out_{u}") for u in range(8)]
            warm_in = dram.tile([256, 512], BF16)
            warm_out = [dram.tile([1024, 512], BF16, name=f"warm_out_{k}")
                        for k in range(2)]
            rc_dram = dram.tile([32, 512], F32)   # recip bounce: [2*(4c+m)+half, q]

            # ---- persistent sbuf ----
            wq_sb = persist.tile([128, 16, 512], BF16)   # [h_in, h_chunk, qd]
            wk_sb = persist.tile([128, 16, 128], BF16)
            wv_sb = persist.tile([128, 16, 128], BF16)
            wo_sb = persist.tile([128, 16, 512], BF16)   # [d_in, d_chunk, hcol]
            em_sb = persist.tile([128, 16, 512], BF16)   # exp(mask)^T per kv tile
            qt_sb = persist.tile([128, 4, S], BF16)      # [qd in pair, pair m, s]
            kt_sb = persist.tile([128, S], BF16)         # [d (2 heads), skv]
            v_sb = persist.tile([128, 16, 130], BF16)    # [skv in tile, t, V0|1|V1|1]
            nc.vector.memset(v_sb, 1.0)

            # warm up the collective rings with two real-size AllGathers
            # (the first large collective on cold rings is 3-10x slower)
            warm_sb = persist.tile([128, 2, 512], BF16)
            nc.vector.memset(warm_sb, 0.0)
            nc.gpsimd.dma_start(
                out=warm_in.rearrange("(mm p) s -> p mm s", p=128),
                in_=warm_sb[:])
            for k in range(2):
                nc.gpsimd.collective_compute(
                    "AllGather", mybir.AluOpType.bypass,
                    replica_groups=[[0, 1, 2, 3], [4, 5, 6, 7]],
                    ins=[warm_in.opt()], outs=[warm_out[k].opt()])

            # ---- phase P: all projections, dense on PE ----
            # xt lives only in this scope; its 64KB/partition is reused by
            # the phase A/O pools afterwards.
            with (
                tc.tile_pool(name="xtpool", bufs=1) as xtpool,
                tc.tile_pool(name="proj_ps", bufs=3, space="PSUM") as proj_ps,
                tc.tile_pool(name="v_ps", bufs=2, space="PSUM") as v_ps,
            ):
                xt = [xtpool.tile([128, S], BF16, name=f"xt_{i}") for i in range(16)]
                # input DMAs split across both HWDGE queues
                nc.scalar.dma_start(out=wq_sb,
                                    in_=wq_in.rearrange("(i p) d -> p i d", p=128))
                for i in range(16):
                    eng = nc.sync if i % 2 == 0 else nc.scalar
                    eng.dma_start(out=xt[i], in_=xT_in[128 * i:128 * (i + 1), :])
                nc.sync.dma_start(out=wk_sb,
                                  in_=wk_in.rearrange("(i p) d -> p i d", p=128))
                nc.sync.dma_start(out=wv_sb,
                                  in_=wv_in.rearrange("(i p) d -> p i d", p=128))
                nc.scalar.dma_start(out=em_sb, in_=em_in)
                nc.sync.dma_start(out=wo_sb,
                                  in_=wo_in.rearrange("(i p) d -> p i d", p=128))
                for c in range(4):
                    cs = slice(512 * c, 512 * (c + 1))
                    for m in range(4):
                        qp = proj_ps.tile([128, 512], F32, tag="pps", name=f"qp_{c}_{m}")
                        for i in range(16):
                            nc.tensor.matmul(qp[:], wq_sb[:, i, 128 * m:128 * (m + 1)],
                                             xt[i][:, cs], start=(i == 0), stop=(i == 15))
                        nc.vector.tensor_copy(qt_sb[:, m, cs], qp[:])
                    kp = proj_ps.tile([128, 512], F32, tag="pps", name=f"kp_{c}")
                    for i in range(16):
                        nc.tensor.matmul(kp[:], wk_sb[:, i, :], xt[i][:, cs],
                                         start=(i == 0), stop=(i == 15))
                    nc.vector.tensor_copy(kt_sb[:, cs], kp[:])
                    for r in range(4):
                        t = 4 * c + r
                        vp = v_ps.tile([128, 128], F32, tag="vps", name=f"vp_{t}")
                        for i in range(16):
                            nc.tensor.matmul(vp[:], xt[i][:, 128 * t:128 * (t + 1)],
                                             wv_sb[:, i, :], start=(i == 0), stop=(i == 15))
                        nc.vector.tensor_copy(v_sb[:, t, 0:64], vp[:, 0:64])
                        nc.vector.tensor_copy(v_sb[:, t, 65:129], vp[:, 64:128])

            # ---- phase A + O: attention, chunked collective, output proj ----
            with contextlib.ExitStack() as ctx:
                sc_ps = ctx.enter_context(tc.tile_pool(name="sc_ps", bufs=2, space="PSUM"))
                av_ps = ctx.enter_context(tc.tile_pool(name="av_ps", bufs=2, space="PSUM"))
                o_ps = ctx.enter_context(tc.tile_pool(name="o_ps", bufs=2, space="PSUM"))
                bc_pool = ctx.enter_context(tc.tile_pool(name="bc", bufs=2))
                probs = ctx.enter_context(tc.tile_pool(name="probs", bufs=4))
                smalls = ctx.enter_context(tc.tile_pool(name="smalls", bufs=2))
                tmp_pool = ctx.enter_context(tc.tile_pool(name="tmp", bufs=2))
                ot_pool = ctx.enter_context(tc.tile_pool(name="ot", bufs=2))
                of_pool = ctx.enter_context(tc.tile_pool(name="of", bufs=2))
                outst = ctx.enter_context(tc.tile_pool(name="outst", bufs=2))

                partial_pool = ctx.enter_context(tc.tile_pool(name="partial", bufs=2))

                # filler FIFO: O-projection PE work pumped into the exp-wait
                # bubbles of the attention tile loop (keeps HAM warm)
                filler = []

                def pump(n):
                    for _ in range(n):
                        if filler:
                            filler.pop(0)()

                def enqueue_oproj_half(c, of_tile, h2, partial):
                    """32 matmuls (+tails) computing the i in [8*h2, 8*h2+8)
                    half of chunk c's output projection."""
                    state = {}
                    for st in range(4):
                        for k in range(8):
                            def mm(st=st, k=k, i=8 * h2 + 0 + 0):
                                i = 8 * h2 + k
                                if k == 0:
                                    state[st] = o_ps.tile(
                                        [128, 512], F32, tag="ops",
                                        name=f"op_{c}_{h2}_{st}")
                                nc.tensor.matmul(
                                    state[st][:],
                                    of_tile[:, i, 128 * st:128 * (st + 1)],
                                    wo_sb[:, i, :],
                                    start=(k == 0), stop=(k == 7))
                            filler.append(mm)

                        if h2 == 0:
                            def tail0(st=st):
                                nc.vector.tensor_copy(partial[:, st, :],
                                                      state[st][:])
                            filler.append(tail0)
                        else:
                            def tail1(st=st):
                                ost = outst.tile([128, 512], F32, tag="ost",
                                                 name=f"ost_{c}_{st}")
                                nc.vector.tensor_tensor(
                                    ost[:], state[st][:], partial[:, st, :],
                                    op=mybir.AluOpType.add)
                                nc.gpsimd.dma_start(
                                    out=out_ext[512 * c + 128 * st:
                                                512 * c + 128 * (st + 1), :],
                                    in_=ost[:])
                            filler.append(tail1)

                def emit_scores(c, m, t):
                    """Scores + fused-pair exp (+ diag mask) for kv tile t."""
                    cs = slice(512 * c, 512 * (c + 1))
                    sp = sc_ps.tile([128, 1024], F32, tag="sp2",
                                    name=f"sp_{c}_{m}_{t}")
                    for half in (0, 1):
                        hp = slice(64 * half, 64 * half + 64)
                        nc.tensor.matmul(sp[:, 512 * half:512 * (half + 1)],
                                         kt_sb[hp, 128 * t:128 * (t + 1)],
                                         qt_sb[hp, m, cs], start=True, stop=True,
                                         tile_position=(64 * half, 0))
                    pr = probs.tile([128, 1024], BF16, tag="pr",
                                    name=f"pr_{c}_{m}_{t}")
                    nc.scalar.activation(pr[:], sp[:],
                                         mybir.ActivationFunctionType.Exp,
                                         scale=SCALE)
                    if t >= 4 * c:
                        nc.vector.tensor_mul(pr[:, 0:512], pr[:, 0:512],
                                             em_sb[:, t, :])
                        nc.vector.tensor_mul(pr[:, 512:1024], pr[:, 512:1024],
                                             em_sb[:, t, :])
                    return pr

                def emit_av(avA, avB, pr, t, first, last):
                    nc.tensor.matmul(avA[:], v_sb[:, t, 0:65], pr[:, 0:512],
                                     start=first, stop=last)
                    nc.tensor.matmul(avB[:], v_sb[:, t, 65:130], pr[:, 512:1024],
                                     start=first, stop=last)

                def attention_chunk(c, ot_tile, of_tile, enq):
                    for m in range(4):
                        for fn in enq.get(m, []):
                            fn()
                        ntile = 4 * c + 4
                        avA = av_ps.tile([65, 512], F32, tag="av", name=f"avA_{c}_{m}")
                        avB = av_ps.tile([65, 512], F32, tag="av", name=f"avB_{c}_{m}")
                        pend = None
                        for t in range(ntile):
                            pr = emit_scores(c, m, t)
                            if pend is not None:
                                pt, ppr = pend
                                emit_av(avA, avB, ppr, pt, pt == 0, False)
                            pend = (t, pr)
                            pump(2)
                        pt, ppr = pend
                        emit_av(avA, avB, ppr, pt, pt == 0, True)

                        # normalization, all off the PE/scalar critical path:
                        # copy av + sum rows out (vector), DMA-bounce broadcast
                        # the sums, then divide on gpsimd
                        sums = smalls.tile([33, 512], F32, tag="rc",
                                           name=f"sums_{c}_{m}")
                        tmp = tmp_pool.tile([128, 512], F32, tag="tmp",
                                            name=f"tmp_{c}_{m}")
                        bc = bc_pool.tile([128, 512], F32, tag="bc",
                                          name=f"bc_{c}_{m}")
                        rcp = smalls.tile([33, 512], F32, tag="rcp",
                                          name=f"rcp_{c}_{m}")
                        nc.vector.tensor_copy(sums[0:1, :], avA[64:65, :])
                        nc.vector.tensor_copy(sums[32:33, :], avB[64:65, :])
                        nc.vector.reciprocal(rcp[:], sums[:])
                        nc.vector.tensor_copy(tmp[0:64, :], avA[0:64, :])
                        nc.vector.tensor_copy(tmp[64:128, :], avB[0:64, :])
                        u = 2 * (4 * c + m)
                        nc.gpsimd.dma_start(out=rc_dram[u:u + 1, :], in_=rcp[0:1, :])
                        nc.gpsimd.dma_start(out=rc_dram[u + 1:u + 2, :],
                                            in_=rcp[32:33, :])
                        nc.gpsimd.dma_start(
                            out=bc[0:64, :],
                            in_=rc_dram[u:u + 1, :].partition_broadcast(64))
                        nc.gpsimd.dma_start(
                            out=bc[64:128, :],
                            in_=rc_dram[u + 1:u + 2, :].partition_broadcast(64))
                        nc.gpsimd.tensor_mul(ot_tile[:, m, :], tmp[:], bc[:])

                        # per-half-chunk collective: gather these 2 head
                        # pairs' OT across the group, load into of
                        if m in (1, 3):
                            h2 = m // 2
                            uu = 2 * c + h2
                            nc.gpsimd.dma_start(
                                out=cc_in[uu].rearrange("(mm p) s -> p mm s", p=128),
                                in_=ot_tile[:, 2 * h2:2 * (h2 + 1), :])
                            nc.gpsimd.collective_compute(
                                "AllGather", mybir.AluOpType.bypass,
                                replica_groups=[[0, 1, 2, 3], [4, 5, 6, 7]],
                                ins=[cc_in[uu].opt()], outs=[cc_out[uu].opt()])
                            nc.sync.dma_start(
                                out=of_tile[:, 8 * h2:8 * (h2 + 1), :],
                                in_=cc_out[uu].rearrange("(r p) s -> p r s", p=128))

                ctxs = {}
                for c in range(4):
                    ot_tile = ot_pool.tile([128, 4, 512], BF16, tag="ot", name=f"ot_{c}")
                    of_tile = of_pool.tile([128, 16, 512], BF16, tag="of",
                                           name=f"of_{c}")
                    partial = partial_pool.tile([128, 4, 512], F32, tag="part",
                                                name=f"part_{c}")
                    enq = {}
                    if c == 1:
                        enq.setdefault(1, []).append(
                            lambda p=ctxs[0]: enqueue_oproj_half(0, p[0], 0, p[1]))
                    if c >= 1:
                        enq.setdefault(1, []).append(
                            lambda p=ctxs[c - 1], cc=c - 1:
                            enqueue_oproj_half(cc, p[0], 1, p[1]))
                        enq.setdefault(3, []).append(
                            lambda cc=c, o=of_tile, pa=partial:
                            enqueue_oproj_half(cc, o, 0, pa))
                    attention_chunk(c, ot_tile, of_tile, enq)
                    ctxs[c] = (of_tile, partial)
                pump(len(filler))
                enqueue_oproj_half(3, ctxs[3][0], 1, ctxs[3][1])
                pump(len(filler))

    _split_excess_waits(nc)
    return nc


def _split_excess_waits(nc, cap=1):
    """Walrus allows few sync-wait slots per instruction; move excess waits
    onto same-engine NoOps placed immediately before (program order keeps
    semantics)."""
    nid = [0]
    for fn in nc.m.functions:
        for bb in fn.blocks:
            insts = list(bb.instructions)
            out = []
            for inst in insts:
                si = getattr(inst, "sync_info", None)
                waits = list(si.on_wait) if si and si.on_wait else []
                if len(waits) > cap:
                    keep = waits[:cap]
                    rest = waits[cap:]
                    while rest:
                        chunk, rest = rest[:cap], rest[cap:]
                        nid[0] += 1
                        nop = mybir.InstNoOp(
                            name=f"waitsplit-{nid[0]}", engine=inst.engine,
                            ins=[], outs=[], bass_nofuse=True,
                            sync_info=mybir.SyncInfo(on_wait=chunk, on_update=[]))
                        out.append(nop)
                    si.on_wait = keep
                out.append(inst)
            bb.instructions[:] = out


def _perm():
    """wo column permutation: gathered-OT row g -> original Wo column.

    Per-half-chunk AllGather: g = 1024*h2 + 256*rank + 128*mm + 64*half + d,
    head pair m = 2*h2 + mm."""
    p = np.zeros(2048, np.int64)
    for g in range(2048):
        h2, r1 = divmod(g, 1024)
        rank, r2 = divmod(r1, 256)
        mm, rr = divmod(r2, 128)
        half, d = divmod(rr, 64)
        h = 8 * rank + 2 * h2 + mm + 4 * half
        p[g] = 64 * h + d
    return p


def kernel(hidden_states, attention_mask, Wq, Wk, Wv, Wo):
    hidden_states = np.asarray(hidden_states, np.float32)
    attention_mask = np.asarray(attention_mask, np.float32)
    Wq = np.asarray(Wq, np.float32); Wk = np.asarray(Wk, np.float32)
    Wv = np.asarray(Wv, np.float32); Wo = np.asarray(Wo, np.float32)

    if "nc" not in _program_cache:
        _program_cache["nc"] = _build_program()
    nc = _program_cache["nc"]

    perm = _perm()
    # per-batch shared tensors
    xTs, ems = [], []
    for bi in range(B):
        xTs.append(np.ascontiguousarray(hidden_states[bi].T).astype(BF16_NP))
        em = np.zeros((128, 16, 512), np.float32)
        for c in range(4):
            blk = attention_mask[bi, 0, 512 * c:512 * (c + 1), 512 * c:512 * (c + 1)]
            emb = np.exp(blk).T  # [kv, q]
            for dd in range(4):
                em[:, 4 * c + dd, :] = emb[128 * dd:128 * (dd + 1), :]
        ems.append(em.astype(BF16_NP))

    wqTs, wkTs, wvTs, woTs = [], [], [], []
    for j in range(4):
        rows = []
        for m in range(4):
            rows.append(Wq[64 * (8 * j + m):64 * (8 * j + m) + 64])
            rows.append(Wq[64 * (8 * j + 4 + m):64 * (8 * j + 4 + m) + 64])
        wq_core = np.concatenate(rows, 0)
        wqTs.append(np.ascontiguousarray(wq_core.T).astype(BF16_NP))
        wkTs.append(np.ascontiguousarray(Wk[128 * j:128 * (j + 1)].T).astype(BF16_NP))
        wvTs.append(np.ascontiguousarray(Wv[128 * j:128 * (j + 1)].T).astype(BF16_NP))
        wo_core = Wo[512 * j:512 * (j + 1)][:, perm]
        woTs.append(np.ascontiguousarray(wo_core.T).astype(BF16_NP))

    in_maps = []
    for core in range(8):
        bi, j = divmod(core, 4)
        in_maps.append({
            "xT": xTs[bi], "em": ems[bi],
            "wqT": wqTs[j], "wkT": wkTs[j], "wvT": wvTs[j], "woT": woTs[j],
        })

    global _last_results
    res = run_bass_kernel_spmd(nc, in_maps, list(range(8)), **_trace_opts)
    _last_results = res
    out = np.zeros((B, S, H), np.float32)
    for core in range(8):
        bi, j = divmod(core, 4)
        out[bi, :, 512 * j:512 * (j + 1)] = res.results[core]["out_part"]
    return out


if __name__ == "__main__":
    ins = {
        "hidden_states": np.random.randn(B, S, H).astype(np.float32),
        "attention_mask": np.zeros((B, 1, S, S), np.float32),
        "Wq": np.random.randn(2048, H).astype(np.float32) * H ** -0.5,
        "Wk": np.random.randn(512, H).astype(np.float32) * H ** -0.5,
        "Wv": np.random.randn(512, H).astype(np.float32) * H ** -0.5,
        "Wo": np.random.randn(H, 2048).astype(np.float32) * H ** -0.5,
    }
    o = kernel(**ins)
    print("ran", o.shape, o.dtype)
